# revision 42
# baseline (speedup 1.0000x reference)
"""GQA attention (B=2,S=2048,D=1024,H=16,KH=4,HD=64) + RoPE + causal mask on 8 trn2 cores.

Sharding: core = (batch b, kv-group g).  Each core computes its 4 query heads'
attention against its single KV head and a partial output  O_g @ wo_g  [S, D];
the host sums the 4 partials per batch.

Per-core device pipeline (everything transposed so softmax-sum runs on the PE):
  - host passes x[b]^T so QKV projections contract D on partitions
  - head_dim of wq/wk is permuted on host to [evens, odds] so RoPE is two
    32-row blocks; scores are invariant to a consistent q/k head_dim permutation
  - RoPE: partition-swap copy (DMA) + bf16 DVE muls + gpsimd add with
    [cos,cos,..]/[-sin,+sin,..] tiles (PSUM evacuation on DVE, not ScalarE,
    which is reserved for exp)
  - scores computed transposed  S^T[k, q] = K^T(lhsT) x Q^T(rhs), bf16 matmuls
  - causal mask added INSIDE score PSUM via extra matmul  (-BIG*I) @ staircase01,
    restricted to the 128-col triangular band; diagonal k-tiles jj>=2 are
    column-restricted in scores/mask/exp/AV (those queries are fully masked)
  - softmax without max-subtraction (scores bounded); exp on ScalarE w/ scale=1/8
  - the attention inner loop is software-pipelined: each kt-pair's AV matmuls
    are emitted two steps late, so the exp latency hides behind later score
    matmuls in the in-order PE stream (keeps the PE dense and HAM-warm)
  - AV uses V augmented with a ones column: one accumulating matmul yields both
    O^T[64, q] and the softmax denominator row
  - normalization: 1/denom alternates DVE reciprocal / ScalarE exp(-ln d) to
    balance the two in-order engine streams; partition-broadcast via a
    step-0-partition DMA source AP; multiply into the wo input tile
  - wo projection consumes O^T chunks directly as lhsT; PSUM -> DRAM stores
  - inputs are host-preblocked so every input DMA is contiguous per
    destination partition, emitted in consumer order (wq + x-block0 first)

Measured (neuron-profile NTFF, max over 8 cores): ~224-228us NEFF execution,
vs 249.5us for the previous baseline; rel err 4.57e-3.
"""

import os
import sys

import numpy as np

for _p in ("/opt/trn_rl_repo", "/root/.axon_site/_ro/trn_rl_repo"):
    if os.path.isdir(_p) and _p not in sys.path:
        sys.path.insert(0, _p)

from contextlib import ExitStack

import concourse.bass as bass
import concourse.tile as tile
from concourse import bacc as _bacc
from concourse import mybir
from concourse.bass_utils import run_bass_kernel_spmd

B, S, D = 2, 2048, 1024
H, KH, HD = 16, 4, 64
REP = H // KH          # 4 query heads per kv head
GH = REP               # heads per core
P = 128
QB = 512               # q block (matmul moving free dim)
NKT = S // P           # 16 key tiles
NQB = S // QB          # 4 q blocks
DCH = D // P           # 8 contraction chunks for D
BIG = 30000.0          # pre-scale additive mask magnitude

f32 = mybir.dt.float32
f32r = mybir.dt.float32r
bf16 = mybir.dt.bfloat16

LAST_EXEC_NS = None
LAST_PROFILE = None


def _classify_mask(mask):
    m = np.asarray(mask).reshape(S, S)
    if not m.any():
        return "none"
    tril = np.tril(np.ones((S, S), dtype=bool))
    if (m[tril] == 0.0).all() and (m[~tril] < -1e30).all():
        return "causal"
    return "general"


def _build_nc(mode):
    nc = bass.Bass()
    # inputs pre-arranged on host so every DMA is contiguous per destination
    # partition (strided gathers ran at ~130GB/s and delayed the first matmul)
    xT = nc.declare_dram_parameter("xT", [NQB, P, DCH, QB], bf16, isOutput=False)
    wq = nc.declare_dram_parameter("wq", [P, DCH, GH * HD], bf16, isOutput=False)
    wk = nc.declare_dram_parameter("wk", [P, DCH, 2 * HD], bf16, isOutput=False)
    wv = nc.declare_dram_parameter("wv", [P, DCH, HD], bf16, isOutput=False)
    wo = nc.declare_dram_parameter("wo", [P, 2, D], bf16, isOutput=False)
    cos = nc.declare_dram_parameter("cos", [P, S], bf16, isOutput=False)
    sin = nc.declare_dram_parameter("sin", [P, S], bf16, isOutput=False)
    stair = nc.declare_dram_parameter("stair", [P, 896], bf16, isOutput=False)
    negI = nc.declare_dram_parameter("negI", [P, P], bf16, isOutput=False)
    ones1 = nc.declare_dram_parameter("ones1", [1, HD], bf16, isOutput=False)
    if mode == "general":
        maskT = nc.declare_dram_parameter("maskT", [NKT, P, S], f32, isOutput=False)
    out = nc.declare_dram_parameter("out", [S, D], f32, isOutput=True)

    with tile.TileContext(nc) as tc, ExitStack() as ctx:
        const = ctx.enter_context(tc.tile_pool(name="const", bufs=1))
        big = ctx.enter_context(tc.tile_pool(name="big", bufs=1))
        work = ctx.enter_context(tc.tile_pool(name="work", bufs=6))
        ptp = ctx.enter_context(tc.tile_pool(name="ptp", bufs=8))
        psp = ctx.enter_context(tc.tile_pool(name="psp", bufs=2, space="PSUM"))
        stp = ctx.enter_context(tc.tile_pool(name="stp", bufs=2, space="PSUM"))
        avp = ctx.enter_context(tc.tile_pool(name="avp", bufs=2, space="PSUM"))

        # ---- constants / weights to SBUF, in consumer order: the first
        # Q-projection matmul only needs wq + x-block0 (+ rope tables). Few
        # DMAs per tile: consumers wait per DMA-queue semaphore, and walrus
        # rejects instructions with too many wait conditions
        xt_sb = big.tile([P, NQB, DCH, QB], bf16, tag="xt")
        wq_sb = const.tile([P, DCH, GH * HD], bf16, tag="wq")
        wk_sb = const.tile([P, DCH, 2 * HD], bf16, tag="wk")
        wv_sb = const.tile([P, DCH, HD], bf16, tag="wv")
        wo_sb = const.tile([P, 2, D], bf16, tag="wo")
        cos_sb = const.tile([P, S], bf16, tag="cos")
        sin_sb = const.tile([P, S], bf16, tag="sin")
        stair_sb = const.tile([P, 896], bf16, tag="stair")
        negI_sb = const.tile([P, P], bf16, tag="negI")
        ones_sb = const.tile([1, HD], bf16, tag="ones1")

        def dma_x_block(sb):
            nc.sync.dma_start(out=xt_sb[:, sb], in_=xT[sb])

        nc.sync.dma_start(out=wq_sb, in_=wq[:, :, :])
        dma_x_block(0)
        nc.sync.dma_start(out=wk_sb, in_=wk[:, :, :])
        nc.sync.dma_start(out=cos_sb, in_=cos[:, :])
        nc.sync.dma_start(out=sin_sb, in_=sin[:, :])
        nc.sync.dma_start(out=wv_sb, in_=wv[:, :, :])
        nc.sync.dma_start(out=stair_sb, in_=stair[:, :])
        nc.sync.dma_start(out=negI_sb, in_=negI[:, :])
        nc.sync.dma_start(out=ones_sb, in_=ones1[:, :])

        # per-s-block tiles: fine-grained deps let attention start as soon as
        # the first s-block of Q/K/V is ready instead of after all of stage A
        QT_t = [big.tile([P, 2, QB], bf16, tag=f"QT{i}", name=f"QT{i}") for i in range(NQB)]
        KT_t = [big.tile([P, QB], bf16, tag=f"KT{i}", name=f"KT{i}") for i in range(NQB)]
        V_t = [big.tile([P, 4, HD + 1], bf16, tag=f"V{i}", name=f"V{i}") for i in range(NQB)]
        OTC_t = [big.tile([P, 2, QB], bf16, tag=f"OTC{i}", name=f"OTC{i}") for i in range(NQB)]
        for i in range(NQB):
            nc.vector.memset(V_t[i][:, :, HD:HD + 1], 1.0)
        ones_f32 = const.tile([1, HD], f32, tag="ones_f32")
        nc.vector.memset(ones_f32, 1.0)

        def rope(ps, out_ap, nrows, sl):
            # ps rows: per 64-group [evens(32), odds(32)]; swap 32-row halves.
            # DMA cannot read PSUM, so evacuate via DVE copy first (casting to
            # bf16 so the swap DMA + muls run in 16-bit DVE modes; the Scalar
            # engine is reserved for exp).
            sb_ps = work.tile([P, QB], bf16, tag="ropesb")
            nc.vector.tensor_copy(sb_ps[:nrows], ps[:nrows])
            tmp = work.tile([P, QB], bf16, tag="ropetmp")
            # swap DMAs ride the gpsimd SWDGE queue: the sync HWDGE queue
            # serializes input loads + out stores, and these 64KB moves were
            # queuing behind them on the stage-A critical path
            for r0 in range(0, nrows, 64):
                nc.gpsimd.dma_start(
                    out=tmp[r0:r0 + 32, :], in_=sb_ps[r0 + 32:r0 + 64, :])
                nc.gpsimd.dma_start(
                    out=tmp[r0 + 32:r0 + 64, :], in_=sb_ps[r0:r0 + 32, :])
            ta = work.tile([P, QB], bf16, tag="ropeta")
            tb = work.tile([P, QB], bf16, tag="ropetb")
            nc.vector.tensor_mul(ta[:nrows], sb_ps[:nrows], cos_sb[:nrows, sl])
            nc.vector.tensor_mul(tb[:nrows], tmp[:nrows], sin_sb[:nrows, sl])
            nc.gpsimd.tensor_add(out_ap, ta[:nrows], tb[:nrows])

        # ---- Q/K/V per s-block (emission order lets qb0 attention start early)
        def emit_stage_a_sb(sb):
          if True:
              sl = slice(sb * QB, (sb + 1) * QB)
              # K first: attention on this block needs KT + QT(ch0) before
              # anything else
              ps = psp.tile([P, QB], f32, tag="proj")
              for dc in range(DCH):
                  nc.tensor.matmul(
                      ps, lhsT=wk_sb[:, dc, :], rhs=xt_sb[:, sb, dc, :],
                      start=(dc == 0), stop=(dc == DCH - 1),
                  )
              rope(ps, KT_t[sb], P, sl)

              def _q_proj(ch):
                  ps = psp.tile([P, QB], f32, tag="proj")
                  for dc in range(DCH):
                      nc.tensor.matmul(
                          ps, lhsT=wq_sb[:, dc, ch * P:(ch + 1) * P],
                          rhs=xt_sb[:, sb, dc, :],
                          start=(dc == 0), stop=(dc == DCH - 1),
                      )
                  rope(ps, QT_t[sb][:, ch, :], P, sl)

              # Q(ch0) before V so attention's first scores start early; V
              # before Q(ch1) so the V casts land early in the in-order DVE
              # stream (they gate AV matmuls and were observed queuing behind
              # reciprocals)
              _q_proj(0)
              for st_i in range(4 * sb, 4 * sb + 4):
                  ps = psp.tile([P, HD], f32, tag="proj")
                  for dc in range(DCH):
                      nc.tensor.matmul(
                          ps, lhsT=xt_sb[:, sb, dc,
                                 (st_i % 4) * P:(st_i % 4 + 1) * P],
                          rhs=wv_sb[:, dc, :],
                          start=(dc == 0), stop=(dc == DCH - 1),
                      )
                  nc.vector.tensor_copy(V_t[sb][:, st_i - 4 * sb, 0:HD], ps)
              _q_proj(1)

        # ---- attention per (head, q block), emitted in wavefront order ----
        def emit_attn(qb, heads=None):
            # Software-pipelined across kt-pairs AND heads: each kt-pair's AV
            # matmuls are emitted two steps late, so the exp latency (~1.1us
            # on ScalarE) hides behind later score matmuls in the in-order PE
            # stream -- including across head boundaries, so the pipeline
            # never drains mid-group. Each head's normalize chain is emitted
            # right after its last AV drain.
            hs = list(heads) if heads is not None else list(range(GH))
            q0 = qb * QB
            qsl = slice(q0, q0 + QB)
            nk = 4 * (qb + 1) if mode == "causal" else NKT
            avs = {}
            pend = []

            def _normalize(h):
                ch, hr = h // 2, (h % 2) * 64
                av = avs[h]
                # r = 1/denom; replicate across 64 partitions with a
                # partition-step-0 DMA source AP (no PE matmul, no PSUM
                # bank). Reciprocal alternates DVE (iterative divide, ~3.4us
                # on one lane) / ACT (1/d = exp(-ln d), 2 spline ops ~0.6us)
                # to balance the two in-order engine streams.
                r1 = work.tile([1, QB], f32, tag="r1", name=f"r1_{qb}_{h}")
                if (4 * qb + h) % 2 == 0:
                    nc.vector.reciprocal(r1, av[HD:HD + 1, :])
                else:
                    lt = work.tile([1, QB], f32, tag="lnt",
                                   name=f"lnt{qb}_{h}")
                    nc.scalar.activation(
                        lt, av[HD:HD + 1, :], mybir.ActivationFunctionType.Ln)
                    nc.scalar.activation(
                        r1, lt, mybir.ActivationFunctionType.Exp, scale=-1.0)
                rbs = work.tile([64, QB], f32, tag="rbs", name=f"rbs{qb}_{h}")
                r1b = bass.AP(tensor=r1.tensor, offset=r1.offset,
                              ap=[list(r1.ap[0]), [0, 64]]
                              + [list(a) for a in r1.ap[1:]])
                with nc.allow_non_contiguous_dma(reason="partition broadcast"):
                    nc.gpsimd.dma_start(out=rbs, in_=r1b)
                ot = work.tile([64, QB], bf16, tag="ot", name=f"ot{qb}_{h}")
                nc.vector.tensor_mul(ot, av[0:HD, :], rbs)
                # gpsimd SWDGE is pinned to one queue -> single wait condition
                # for the wo matmuls that consume OTC
                nc.gpsimd.dma_start(out=OTC_t[qb][hr:hr + 64, ch, :], in_=ot)

            def _drain_av(item):
                h_, pt_, c0s_, kt0_ = item
                for j in range(2):
                    kt = kt0_ + j
                    nc.tensor.matmul(
                        avs[h_][:, c0s_[j]:], lhsT=V_t[kt // 4][:, kt % 4, :],
                        rhs=pt_[:, j, c0s_[j]:],
                        start=(kt == 0), stop=(kt == nk - 1),
                    )
                if kt0_ == nk - 2:
                    _normalize(h_)

            for h in hs:
                ch, hr = h // 2, (h % 2) * 64
                avs[h] = avp.tile([HD + 1, QB], f32, tag="av",
                                  name=f"av{qb}_{h}")
                for kt0 in range(0, nk, 2):
                    st = stp.tile([P, 2, QB], f32, tag="st",
                                  name=f"st{qb}_{h}_{kt0}")
                    # column restriction: on diagonal k-tile jj (= kt-4qb),
                    # queries with q_local < 128*jj are fully masked; for
                    # jj >= 2 skip those columns in scores/mask/exp/AV
                    c0s = [0, 0]
                    for j in range(2):
                        kt = kt0 + j
                        diag = mode == "causal" and kt >= nk - 4
                        jj = kt - (nk - 4)
                        c0 = 128 * jj if (diag and jj >= 2) else 0
                        c0s[j] = c0
                        nc.tensor.matmul(
                            st[:, j, c0:],
                            lhsT=KT_t[kt // 4][hr:hr + 64,
                                               (kt % 4) * P:(kt % 4 + 1) * P],
                            rhs=QT_t[qb][hr:hr + 64, ch, c0:],
                            start=True, stop=not diag,
                        )
                        if diag:
                            # mask needed on [c0, 128*jj+128): below 128*jj
                            # every key is future (fully masked), inside the
                            # 128-col block the staircase applies, and all
                            # later columns are fully unmasked
                            off = 384 - 128 * jj
                            cm = 128 * jj + 128
                            nc.tensor.matmul(
                                st[:, j, c0:cm], lhsT=negI_sb,
                                rhs=stair_sb[:, off + c0:off + cm],
                                start=False, stop=True,
                            )
                        if mode == "general":
                            mt = work.tile([P, QB], f32, tag="maskt")
                            nc.sync.dma_start(out=mt, in_=maskT[kt, :, qsl])
                            nc.vector.tensor_add(st[:, j, :], st[:, j, :], mt)
                    pt = ptp.tile([P, 2, QB], bf16, tag="pt",
                                  name=f"pt{qb}_{h}_{kt0}")
                    if c0s[0] == 0 and c0s[1] == 0:
                        nc.scalar.activation(
                            pt, st, mybir.ActivationFunctionType.Exp, scale=0.125
                        )
                    else:
                        for j in range(2):
                            nc.scalar.activation(
                                pt[:, j, c0s[j]:], st[:, j, c0s[j]:],
                                mybir.ActivationFunctionType.Exp, scale=0.125,
                            )
                    pend.append((h, pt, list(c0s), kt0))
                    if len(pend) > 2:
                        _drain_av(pend.pop(0))
            while pend:
                _drain_av(pend.pop(0))

        # ---- output projection: out[q,:] = sum_c OTC[:,c,q].T @ wo[c] ----
        def emit_wo(qb):
            for qt in range(4 * qb, 4 * qb + 4):
                for dh in range(2):
                    ps = psp.tile([P, 512], f32, tag="proj")
                    for c in range(2):
                        nc.tensor.matmul(
                            ps,
                            lhsT=OTC_t[qt // 4][:, c,
                                                (qt % 4) * P:(qt % 4 + 1) * P],
                            rhs=wo_sb[:, c, dh * 512:(dh + 1) * 512],
                            start=(c == 0), stop=(c == 1),
                        )
                    osb = work.tile([P, 512], f32, tag="osb")
                    nc.vector.tensor_copy(osb, ps)
                    nc.sync.dma_start(
                        out=out[qt * P:(qt + 1) * P,
                                dh * 512:(dh + 1) * 512], in_=osb
                    )

        # offset-by-one interleave: attention for qb emitted after stage-A
        # block qb+1, so projections keep a one-block head start on the PE
        if mode == "causal":
            # attn(qb) only reads KT/V s-blocks <= qb, all emitted beforehand
            emit_stage_a_sb(0)
            dma_x_block(1)
            emit_attn(0, heads=[0, 1])
            emit_stage_a_sb(1)
            dma_x_block(2)
            nc.sync.dma_start(out=wo_sb, in_=wo[:, :, :])
            emit_attn(0, heads=[2, 3])
            emit_attn(1, heads=[0, 1])
            emit_stage_a_sb(2)
            dma_x_block(3)
            emit_attn(1, heads=[2, 3])
            emit_attn(2, heads=[0, 1])
            emit_stage_a_sb(3)
            emit_attn(2, heads=[2, 3])
            emit_attn(3)
        else:
            # non-causal attn reads ALL KT/V tiles: emitting it early would
            # precede their writers (Tile records deps at emission time)
            dma_x_block(1)
            dma_x_block(2)
            dma_x_block(3)
            nc.sync.dma_start(out=wo_sb, in_=wo[:, :, :])
            for _sb in range(NQB):
                emit_stage_a_sb(_sb)
            for _qb in range(NQB):
                emit_attn(_qb)
        for _qb in range(NQB):
            emit_wo(_qb)
    # split multi-wait conditions: TRN2 instructions hold at most one sync
    # wait (EventSemaphore holds two); walrus refuses to split them itself
    import bass_rust
    bass_rust.move_matmul_waits_to_ldweights(nc.m)
    bass_rust.generate_event_semaphores(nc)
    return nc


_NC_CACHE = {}


def _install_ntff_hook():
    """Best-effort: register the axon NTFF profile hook so trace=True can
    measure true NEFF execution time. Harmless no-op when unavailable."""
    try:
        import types
        if "antenv.axon_hooks" in sys.modules:
            return True
        import antenv
        mod = types.ModuleType("antenv.axon_hooks")
        mod._hook = None
        def set_axon_ntff_profile_hook(h):
            mod._hook = h
        def get_axon_ntff_profile_hook():
            return mod._hook
        mod.set_axon_ntff_profile_hook = set_axon_ntff_profile_hook
        mod.get_axon_ntff_profile_hook = get_axon_ntff_profile_hook
        from trn_agent_boot.trn_boot import _ntff_profile_via_ctypes
        hook = _ntff_profile_via_ctypes('/opt/axon/libaxon_pjrt.so')
        if hook is None:
            return False
        mod.set_axon_ntff_profile_hook(hook)
        sys.modules["antenv.axon_hooks"] = mod
        antenv.axon_hooks = mod
        return True
    except Exception:
        return False


def kernel(_trace=False, _trace_cores=None, **inputs):
    global LAST_EXEC_NS, LAST_PROFILE
    x = np.ascontiguousarray(np.asarray(inputs["x"], dtype=np.float32))
    wq = np.asarray(inputs["wq"], dtype=np.float32)
    wk = np.asarray(inputs["wk"], dtype=np.float32)
    wv = np.asarray(inputs["wv"], dtype=np.float32)
    wo = np.asarray(inputs["wo"], dtype=np.float32)
    fc = np.asarray(inputs["freqs_cos"], dtype=np.float32)
    fs = np.asarray(inputs["freqs_sin"], dtype=np.float32)
    mask = np.asarray(inputs["mask"], dtype=np.float32)

    mode = _classify_mask(mask)
    if mode not in _NC_CACHE:
        _NC_CACHE[mode] = _build_nc(mode)
    nc = _NC_CACHE[mode]
    in_maps = _make_in_maps(x, wq, wk, wv, wo, fc, fs, mask, mode)

    if _trace:
        _install_ntff_hook()
    kw = {"trace_cores": _trace_cores} if _trace_cores else {}
    try:
        res = run_bass_kernel_spmd(
            nc, in_maps, core_ids=list(range(8)), trace=_trace, **kw)
    except (ModuleNotFoundError, ImportError):
        res = run_bass_kernel_spmd(
            nc, in_maps, core_ids=list(range(8)), trace=False)
    LAST_EXEC_NS = res.exec_time_ns
    LAST_PROFILE = res.profile_json
    full = np.zeros((B, S, D), dtype=np.float32)
    for b in range(B):
        for g in range(KH):
            full[b] += res.results[b * KH + g]["out"]
    return full


def _make_in_maps(x, wq, wk, wv, wo, fc, fs, mask, mode):
    # head_dim permutation: evens then odds (consistent on q & k -> scores invariant)
    perm = np.concatenate([np.arange(0, HD, 2), np.arange(1, HD, 2)])
    wq_p = wq.reshape(D, H, HD)[:, :, perm].reshape(D, H * HD)
    wk_p = wk.reshape(D, KH, HD)[:, :, perm].reshape(D, KH * HD)

    cosT = fc.T.astype(np.float32)                      # [32, S]
    sinT = fs.T.astype(np.float32)
    cos_rep = np.ascontiguousarray(np.tile(cosT, (4, 1)))          # [128, S]
    sin_signed = np.ascontiguousarray(
        np.concatenate([-sinT, sinT, -sinT, sinT], axis=0))        # [128, S]

    cc = np.arange(P)[:, None]
    mm = np.arange(896)[None, :]
    stair = (cc > (mm - 384)).astype(np.float32)
    negI = (-BIG * np.eye(P)).astype(np.float32)
    ones1 = np.ones((1, HD), dtype=np.float32)

    import ml_dtypes
    b16 = ml_dtypes.bfloat16

    def _pcf(w_slice, width):
        # [D, width] -> [P, DCH, width]: partition-contiguous weight layout
        a = np.ascontiguousarray(w_slice).reshape(DCH, P, width)
        return np.ascontiguousarray(a.transpose(1, 0, 2))

    in_maps = []
    for b in range(B):
        # x[b].T [D, S] -> [sb, p, c, q] so each s-block DMA is contiguous
        xTb = np.ascontiguousarray(
            x[b].T.reshape(DCH, P, NQB, QB).transpose(2, 1, 0, 3)
        ).astype(b16)
        for g in range(KH):
            wk_g = wk_p[:, g * HD:(g + 1) * HD]
            wk_dup = np.concatenate([wk_g, wk_g], axis=1)       # [D, 128]
            wo_g = wo[g * GH * HD:(g + 1) * GH * HD].reshape(2, P, D)
            m = {
                "xT": xTb,
                "wq": _pcf(wq_p[:, g * GH * HD:(g + 1) * GH * HD],
                           GH * HD).astype(b16),
                "wk": _pcf(wk_dup, 2 * HD).astype(b16),
                "wv": _pcf(wv[:, g * HD:(g + 1) * HD], HD).astype(b16),
                "wo": np.ascontiguousarray(
                    wo_g.transpose(1, 0, 2)).astype(b16),
                "cos": cos_rep.astype(b16),
                "sin": sin_signed.astype(b16),
                "stair": stair.astype(b16),
                "negI": negI.astype(b16),
                "ones1": ones1.astype(b16),
            }
            if mode == "general":
                m["maskT"] = np.ascontiguousarray(
                    mask.reshape(S, S).T).reshape(NKT, P, S)
            in_maps.append(m)
    return in_maps



# revision 43
# speedup vs baseline: 1.1481x; 1.1481x over previous
"""GQA attention (B=2,S=2048,D=1024,H=16,KH=4,HD=64) + RoPE + causal mask on 8 trn2 cores.

Sharding: core = (batch b, kv-group g).  Each core computes its 4 query heads'
attention against its single KV head and a partial output  O_g @ wo_g  [S, D];
the host sums the 4 partials per batch.

Per-core device pipeline (everything transposed so softmax-sum runs on the PE):
  - host passes x[b]^T so QKV projections contract D on partitions
  - head_dim of wq/wk is permuted on host to [evens, odds] so RoPE is two
    32-row blocks; scores are invariant to a consistent q/k head_dim permutation
  - RoPE: partition-swap copy (DMA) + bf16 DVE muls + gpsimd add with
    [cos,cos,..]/[-sin,+sin,..] tiles (PSUM evacuation on DVE, not ScalarE,
    which is reserved for exp)
  - scores computed transposed  S^T[k, q] = K^T(lhsT) x Q^T(rhs), bf16 matmuls
  - causal mask added INSIDE score PSUM via extra matmul  (-BIG*I) @ staircase01,
    restricted to the 128-col triangular band; diagonal k-tiles jj>=2 are
    column-restricted in scores/mask/exp/AV (those queries are fully masked)
  - softmax without max-subtraction (scores bounded); exp on ScalarE w/ scale=1/8
  - the attention inner loop is software-pipelined: each kt-pair's AV matmuls
    are emitted two steps late, so the exp latency hides behind later score
    matmuls in the in-order PE stream (keeps the PE dense and HAM-warm)
  - AV uses V augmented with a ones column: one accumulating matmul yields both
    O^T[64, q] and the softmax denominator row
  - normalization: 1/denom alternates DVE reciprocal / ScalarE exp(-ln d) to
    balance the two in-order engine streams; partition-broadcast via a
    step-0-partition DMA source AP; multiply into the wo input tile
  - wo projection consumes O^T chunks directly as lhsT; PSUM -> DRAM stores
  - inputs are host-preblocked so every input DMA is contiguous per
    destination partition, emitted in consumer order (wq + x-block0 first)

Measured (neuron-profile NTFF, max over 8 cores): ~224-228us NEFF execution,
vs 249.5us for the previous baseline; rel err 4.57e-3.
"""

import os
import sys

import numpy as np

for _p in ("/opt/trn_rl_repo", "/root/.axon_site/_ro/trn_rl_repo"):
    if os.path.isdir(_p) and _p not in sys.path:
        sys.path.insert(0, _p)

from contextlib import ExitStack

import concourse.bass as bass
import concourse.tile as tile
from concourse import bacc as _bacc
from concourse import mybir
from concourse.bass_utils import run_bass_kernel_spmd

B, S, D = 2, 2048, 1024
H, KH, HD = 16, 4, 64
REP = H // KH          # 4 query heads per kv head
GH = REP               # heads per core
P = 128
QB = 512               # q block (matmul moving free dim)
NKT = S // P           # 16 key tiles
NQB = S // QB          # 4 q blocks
DCH = D // P           # 8 contraction chunks for D
BIG = 30000.0          # pre-scale additive mask magnitude

f32 = mybir.dt.float32
f32r = mybir.dt.float32r
bf16 = mybir.dt.bfloat16

LAST_EXEC_NS = None
LAST_PROFILE = None


def _classify_mask(mask):
    m = np.asarray(mask).reshape(S, S)
    if not m.any():
        return "none"
    tril = np.tril(np.ones((S, S), dtype=bool))
    if (m[tril] == 0.0).all() and (m[~tril] < -1e30).all():
        return "causal"
    return "general"


def _build_nc(mode):
    nc = bass.Bass()
    # inputs pre-arranged on host so every DMA is contiguous per destination
    # partition (strided gathers ran at ~130GB/s and delayed the first matmul)
    xT = nc.declare_dram_parameter("xT", [NQB, P, DCH, QB], bf16, isOutput=False)
    wq = nc.declare_dram_parameter("wq", [P, DCH, GH * HD], bf16, isOutput=False)
    wk = nc.declare_dram_parameter("wk", [P, DCH, 2 * HD], bf16, isOutput=False)
    wv = nc.declare_dram_parameter("wv", [P, DCH, HD], bf16, isOutput=False)
    wo = nc.declare_dram_parameter("wo", [P, 2, D], bf16, isOutput=False)
    cos = nc.declare_dram_parameter("cos", [P, S], bf16, isOutput=False)
    sin = nc.declare_dram_parameter("sin", [P, S], bf16, isOutput=False)
    stair = nc.declare_dram_parameter("stair", [P, 896], bf16, isOutput=False)
    negI = nc.declare_dram_parameter("negI", [P, P], bf16, isOutput=False)
    ones1 = nc.declare_dram_parameter("ones1", [1, HD], bf16, isOutput=False)
    if mode == "general":
        maskT = nc.declare_dram_parameter("maskT", [NKT, P, S], f32, isOutput=False)
    out = nc.declare_dram_parameter("out", [S, D], f32, isOutput=True)

    with tile.TileContext(nc) as tc, ExitStack() as ctx:
        const = ctx.enter_context(tc.tile_pool(name="const", bufs=1))
        big = ctx.enter_context(tc.tile_pool(name="big", bufs=1))
        work = ctx.enter_context(tc.tile_pool(name="work", bufs=6))
        ptp = ctx.enter_context(tc.tile_pool(name="ptp", bufs=8))
        psp = ctx.enter_context(tc.tile_pool(name="psp", bufs=2, space="PSUM"))
        stp = ctx.enter_context(tc.tile_pool(name="stp", bufs=2, space="PSUM"))
        avp = ctx.enter_context(tc.tile_pool(name="avp", bufs=2, space="PSUM"))

        # ---- constants / weights to SBUF, in consumer order: the first
        # Q-projection matmul only needs wq + x-block0 (+ rope tables). Few
        # DMAs per tile: consumers wait per DMA-queue semaphore, and walrus
        # rejects instructions with too many wait conditions
        xt_sb = big.tile([P, NQB, DCH, QB], bf16, tag="xt")
        wq_sb = const.tile([P, DCH, GH * HD], bf16, tag="wq")
        wk_sb = const.tile([P, DCH, 2 * HD], bf16, tag="wk")
        wv_sb = const.tile([P, DCH, HD], bf16, tag="wv")
        wo_sb = const.tile([P, 2, D], bf16, tag="wo")
        cos_sb = const.tile([P, S], bf16, tag="cos")
        sin_sb = const.tile([P, S], bf16, tag="sin")
        stair_sb = const.tile([P, 896], bf16, tag="stair")
        negI_sb = const.tile([P, P], bf16, tag="negI")
        ones_sb = const.tile([1, HD], bf16, tag="ones1")

        def dma_x_block(sb):
            nc.sync.dma_start(out=xt_sb[:, sb], in_=xT[sb])

        nc.sync.dma_start(out=wq_sb, in_=wq[:, :, :])
        dma_x_block(0)
        nc.sync.dma_start(out=wk_sb, in_=wk[:, :, :])
        nc.sync.dma_start(out=cos_sb, in_=cos[:, :])
        nc.sync.dma_start(out=sin_sb, in_=sin[:, :])
        nc.sync.dma_start(out=wv_sb, in_=wv[:, :, :])
        nc.sync.dma_start(out=stair_sb, in_=stair[:, :])
        nc.sync.dma_start(out=negI_sb, in_=negI[:, :])
        nc.sync.dma_start(out=ones_sb, in_=ones1[:, :])

        # per-s-block tiles: fine-grained deps let attention start as soon as
        # the first s-block of Q/K/V is ready instead of after all of stage A
        QT_t = [big.tile([P, 2, QB], bf16, tag=f"QT{i}", name=f"QT{i}") for i in range(NQB)]
        KT_t = [big.tile([P, QB], bf16, tag=f"KT{i}", name=f"KT{i}") for i in range(NQB)]
        V_t = [big.tile([P, 4, HD + 1], bf16, tag=f"V{i}", name=f"V{i}") for i in range(NQB)]
        OTC_t = [big.tile([P, 2, QB], bf16, tag=f"OTC{i}", name=f"OTC{i}") for i in range(NQB)]
        for i in range(NQB):
            nc.vector.memset(V_t[i][:, :, HD:HD + 1], 1.0)
        ones_f32 = const.tile([1, HD], f32, tag="ones_f32")
        nc.vector.memset(ones_f32, 1.0)

        def rope(ps, out_ap, nrows, sl):
            # ps rows: per 64-group [evens(32), odds(32)]; swap 32-row halves.
            # DMA cannot read PSUM, so evacuate via DVE copy first (casting to
            # bf16 so the swap DMA + muls run in 16-bit DVE modes; the Scalar
            # engine is reserved for exp).
            sb_ps = work.tile([P, QB], bf16, tag="ropesb")
            nc.vector.tensor_copy(sb_ps[:nrows], ps[:nrows])
            tmp = work.tile([P, QB], bf16, tag="ropetmp")
            for r0 in range(0, nrows, 64):
                nc.sync.dma_start(
                    out=tmp[r0:r0 + 32, :], in_=sb_ps[r0 + 32:r0 + 64, :])
                nc.sync.dma_start(
                    out=tmp[r0 + 32:r0 + 64, :], in_=sb_ps[r0:r0 + 32, :])
            ta = work.tile([P, QB], bf16, tag="ropeta")
            tb = work.tile([P, QB], bf16, tag="ropetb")
            nc.vector.tensor_mul(ta[:nrows], sb_ps[:nrows], cos_sb[:nrows, sl])
            nc.vector.tensor_mul(tb[:nrows], tmp[:nrows], sin_sb[:nrows, sl])
            nc.gpsimd.tensor_add(out_ap, ta[:nrows], tb[:nrows])

        # ---- Q/K/V per s-block (emission order lets qb0 attention start early)
        def emit_stage_a_sb(sb):
          if True:
              sl = slice(sb * QB, (sb + 1) * QB)
              # K first: attention on this block needs KT + QT(ch0) before
              # anything else
              ps = psp.tile([P, QB], f32, tag="proj")
              for dc in range(DCH):
                  nc.tensor.matmul(
                      ps, lhsT=wk_sb[:, dc, :], rhs=xt_sb[:, sb, dc, :],
                      start=(dc == 0), stop=(dc == DCH - 1),
                  )
              rope(ps, KT_t[sb], P, sl)

              def _q_proj(ch):
                  ps = psp.tile([P, QB], f32, tag="proj")
                  for dc in range(DCH):
                      nc.tensor.matmul(
                          ps, lhsT=wq_sb[:, dc, ch * P:(ch + 1) * P],
                          rhs=xt_sb[:, sb, dc, :],
                          start=(dc == 0), stop=(dc == DCH - 1),
                      )
                  rope(ps, QT_t[sb][:, ch, :], P, sl)

              # Q(ch0) before V so attention's first scores start early; V
              # before Q(ch1) so the V casts land early in the in-order DVE
              # stream (they gate AV matmuls and were observed queuing behind
              # reciprocals)
              _q_proj(0)
              for st_i in range(4 * sb, 4 * sb + 4):
                  ps = psp.tile([P, HD], f32, tag="proj")
                  for dc in range(DCH):
                      nc.tensor.matmul(
                          ps, lhsT=xt_sb[:, sb, dc,
                                 (st_i % 4) * P:(st_i % 4 + 1) * P],
                          rhs=wv_sb[:, dc, :],
                          start=(dc == 0), stop=(dc == DCH - 1),
                      )
                  nc.vector.tensor_copy(V_t[sb][:, st_i - 4 * sb, 0:HD], ps)
              _q_proj(1)

        # ---- attention per (head, q block), emitted in wavefront order ----
        def emit_attn(qb, heads=None):
            # Software-pipelined across kt-pairs AND heads: each kt-pair's AV
            # matmuls are emitted two steps late, so the exp latency (~1.1us
            # on ScalarE) hides behind later score matmuls in the in-order PE
            # stream -- including across head boundaries, so the pipeline
            # never drains mid-group. Each head's normalize chain is emitted
            # right after its last AV drain.
            hs = list(heads) if heads is not None else list(range(GH))
            q0 = qb * QB
            qsl = slice(q0, q0 + QB)
            nk = 4 * (qb + 1) if mode == "causal" else NKT
            avs = {}
            pend = []

            def _normalize(h):
                ch, hr = h // 2, (h % 2) * 64
                av = avs[h]
                # r = 1/denom; replicate across 64 partitions with a
                # partition-step-0 DMA source AP (no PE matmul, no PSUM
                # bank). Reciprocal alternates DVE (iterative divide, ~3.4us
                # on one lane) / ACT (1/d = exp(-ln d), 2 spline ops ~0.6us)
                # to balance the two in-order engine streams.
                r1 = work.tile([1, QB], f32, tag="r1", name=f"r1_{qb}_{h}")
                if (4 * qb + h) % 2 == 0:
                    nc.vector.reciprocal(r1, av[HD:HD + 1, :])
                else:
                    lt = work.tile([1, QB], f32, tag="lnt",
                                   name=f"lnt{qb}_{h}")
                    nc.scalar.activation(
                        lt, av[HD:HD + 1, :], mybir.ActivationFunctionType.Ln)
                    nc.scalar.activation(
                        r1, lt, mybir.ActivationFunctionType.Exp, scale=-1.0)
                rbs = work.tile([64, QB], f32, tag="rbs", name=f"rbs{qb}_{h}")
                r1b = bass.AP(tensor=r1.tensor, offset=r1.offset,
                              ap=[list(r1.ap[0]), [0, 64]]
                              + [list(a) for a in r1.ap[1:]])
                with nc.allow_non_contiguous_dma(reason="partition broadcast"):
                    nc.gpsimd.dma_start(out=rbs, in_=r1b)
                ot = work.tile([64, QB], bf16, tag="ot", name=f"ot{qb}_{h}")
                nc.vector.tensor_mul(ot, av[0:HD, :], rbs)
                # gpsimd SWDGE is pinned to one queue -> single wait condition
                # for the wo matmuls that consume OTC
                nc.gpsimd.dma_start(out=OTC_t[qb][hr:hr + 64, ch, :], in_=ot)

            def _drain_av(item):
                h_, pt_, c0s_, kt0_ = item
                for j in range(2):
                    kt = kt0_ + j
                    nc.tensor.matmul(
                        avs[h_][:, c0s_[j]:], lhsT=V_t[kt // 4][:, kt % 4, :],
                        rhs=pt_[:, j, c0s_[j]:],
                        start=(kt == 0), stop=(kt == nk - 1),
                    )
                if kt0_ == nk - 2:
                    _normalize(h_)

            for h in hs:
                ch, hr = h // 2, (h % 2) * 64
                avs[h] = avp.tile([HD + 1, QB], f32, tag="av",
                                  name=f"av{qb}_{h}")
                for kt0 in range(0, nk, 2):
                    st = stp.tile([P, 2, QB], f32, tag="st",
                                  name=f"st{qb}_{h}_{kt0}")
                    # column restriction: on diagonal k-tile jj (= kt-4qb),
                    # queries with q_local < 128*jj are fully masked; for
                    # jj >= 2 skip those columns in scores/mask/exp/AV
                    c0s = [0, 0]
                    for j in range(2):
                        kt = kt0 + j
                        diag = mode == "causal" and kt >= nk - 4
                        jj = kt - (nk - 4)
                        c0 = 128 * jj if (diag and jj >= 2) else 0
                        c0s[j] = c0
                        nc.tensor.matmul(
                            st[:, j, c0:],
                            lhsT=KT_t[kt // 4][hr:hr + 64,
                                               (kt % 4) * P:(kt % 4 + 1) * P],
                            rhs=QT_t[qb][hr:hr + 64, ch, c0:],
                            start=True, stop=not diag,
                        )
                        if diag:
                            # mask needed on [c0, 128*jj+128): below 128*jj
                            # every key is future (fully masked), inside the
                            # 128-col block the staircase applies, and all
                            # later columns are fully unmasked
                            off = 384 - 128 * jj
                            cm = 128 * jj + 128
                            nc.tensor.matmul(
                                st[:, j, c0:cm], lhsT=negI_sb,
                                rhs=stair_sb[:, off + c0:off + cm],
                                start=False, stop=True,
                            )
                        if mode == "general":
                            mt = work.tile([P, QB], f32, tag="maskt")
                            nc.sync.dma_start(out=mt, in_=maskT[kt, :, qsl])
                            nc.vector.tensor_add(st[:, j, :], st[:, j, :], mt)
                    pt = ptp.tile([P, 2, QB], bf16, tag="pt",
                                  name=f"pt{qb}_{h}_{kt0}")
                    if c0s[0] == 0 and c0s[1] == 0:
                        nc.scalar.activation(
                            pt, st, mybir.ActivationFunctionType.Exp, scale=0.125
                        )
                    else:
                        for j in range(2):
                            nc.scalar.activation(
                                pt[:, j, c0s[j]:], st[:, j, c0s[j]:],
                                mybir.ActivationFunctionType.Exp, scale=0.125,
                            )
                    pend.append((h, pt, list(c0s), kt0))
                    if len(pend) > 2:
                        _drain_av(pend.pop(0))
            while pend:
                _drain_av(pend.pop(0))

        # ---- output projection: out[q,:] = sum_c OTC[:,c,q].T @ wo[c] ----
        def emit_wo(qb):
            for qt in range(4 * qb, 4 * qb + 4):
                for dh in range(2):
                    ps = psp.tile([P, 512], f32, tag="proj")
                    for c in range(2):
                        nc.tensor.matmul(
                            ps,
                            lhsT=OTC_t[qt // 4][:, c,
                                                (qt % 4) * P:(qt % 4 + 1) * P],
                            rhs=wo_sb[:, c, dh * 512:(dh + 1) * 512],
                            start=(c == 0), stop=(c == 1),
                        )
                    osb = work.tile([P, 512], f32, tag="osb")
                    nc.vector.tensor_copy(osb, ps)
                    nc.sync.dma_start(
                        out=out[qt * P:(qt + 1) * P,
                                dh * 512:(dh + 1) * 512], in_=osb
                    )

        # offset-by-one interleave: attention for qb emitted after stage-A
        # block qb+1, so projections keep a one-block head start on the PE
        if mode == "causal":
            # attn(qb) only reads KT/V s-blocks <= qb, all emitted beforehand
            emit_stage_a_sb(0)
            dma_x_block(1)
            emit_attn(0, heads=[0, 1])
            emit_stage_a_sb(1)
            dma_x_block(2)
            nc.sync.dma_start(out=wo_sb, in_=wo[:, :, :])
            emit_attn(0, heads=[2, 3])
            emit_attn(1, heads=[0, 1])
            emit_stage_a_sb(2)
            dma_x_block(3)
            emit_attn(1, heads=[2, 3])
            emit_attn(2, heads=[0, 1])
            emit_stage_a_sb(3)
            emit_attn(2, heads=[2, 3])
            emit_attn(3)
        else:
            # non-causal attn reads ALL KT/V tiles: emitting it early would
            # precede their writers (Tile records deps at emission time)
            dma_x_block(1)
            dma_x_block(2)
            dma_x_block(3)
            nc.sync.dma_start(out=wo_sb, in_=wo[:, :, :])
            for _sb in range(NQB):
                emit_stage_a_sb(_sb)
            for _qb in range(NQB):
                emit_attn(_qb)
        for _qb in range(NQB):
            emit_wo(_qb)
    # split multi-wait conditions: TRN2 instructions hold at most one sync
    # wait (EventSemaphore holds two); walrus refuses to split them itself
    import bass_rust
    bass_rust.move_matmul_waits_to_ldweights(nc.m)
    bass_rust.generate_event_semaphores(nc)
    return nc


_NC_CACHE = {}


def _install_ntff_hook():
    """Best-effort: register the axon NTFF profile hook so trace=True can
    measure true NEFF execution time. Harmless no-op when unavailable."""
    try:
        import types
        if "antenv.axon_hooks" in sys.modules:
            return True
        import antenv
        mod = types.ModuleType("antenv.axon_hooks")
        mod._hook = None
        def set_axon_ntff_profile_hook(h):
            mod._hook = h
        def get_axon_ntff_profile_hook():
            return mod._hook
        mod.set_axon_ntff_profile_hook = set_axon_ntff_profile_hook
        mod.get_axon_ntff_profile_hook = get_axon_ntff_profile_hook
        from trn_agent_boot.trn_boot import _ntff_profile_via_ctypes
        hook = _ntff_profile_via_ctypes('/opt/axon/libaxon_pjrt.so')
        if hook is None:
            return False
        mod.set_axon_ntff_profile_hook(hook)
        sys.modules["antenv.axon_hooks"] = mod
        antenv.axon_hooks = mod
        return True
    except Exception:
        return False


def kernel(_trace=False, _trace_cores=None, **inputs):
    global LAST_EXEC_NS, LAST_PROFILE
    x = np.ascontiguousarray(np.asarray(inputs["x"], dtype=np.float32))
    wq = np.asarray(inputs["wq"], dtype=np.float32)
    wk = np.asarray(inputs["wk"], dtype=np.float32)
    wv = np.asarray(inputs["wv"], dtype=np.float32)
    wo = np.asarray(inputs["wo"], dtype=np.float32)
    fc = np.asarray(inputs["freqs_cos"], dtype=np.float32)
    fs = np.asarray(inputs["freqs_sin"], dtype=np.float32)
    mask = np.asarray(inputs["mask"], dtype=np.float32)

    mode = _classify_mask(mask)
    if mode not in _NC_CACHE:
        _NC_CACHE[mode] = _build_nc(mode)
    nc = _NC_CACHE[mode]
    in_maps = _make_in_maps(x, wq, wk, wv, wo, fc, fs, mask, mode)

    if _trace:
        _install_ntff_hook()
    kw = {"trace_cores": _trace_cores} if _trace_cores else {}
    try:
        res = run_bass_kernel_spmd(
            nc, in_maps, core_ids=list(range(8)), trace=_trace, **kw)
    except (ModuleNotFoundError, ImportError):
        res = run_bass_kernel_spmd(
            nc, in_maps, core_ids=list(range(8)), trace=False)
    LAST_EXEC_NS = res.exec_time_ns
    LAST_PROFILE = res.profile_json
    full = np.zeros((B, S, D), dtype=np.float32)
    for b in range(B):
        for g in range(KH):
            full[b] += res.results[b * KH + g]["out"]
    return full


def _make_in_maps(x, wq, wk, wv, wo, fc, fs, mask, mode):
    # head_dim permutation: evens then odds (consistent on q & k -> scores invariant)
    perm = np.concatenate([np.arange(0, HD, 2), np.arange(1, HD, 2)])
    wq_p = wq.reshape(D, H, HD)[:, :, perm].reshape(D, H * HD)
    wk_p = wk.reshape(D, KH, HD)[:, :, perm].reshape(D, KH * HD)

    cosT = fc.T.astype(np.float32)                      # [32, S]
    sinT = fs.T.astype(np.float32)
    cos_rep = np.ascontiguousarray(np.tile(cosT, (4, 1)))          # [128, S]
    sin_signed = np.ascontiguousarray(
        np.concatenate([-sinT, sinT, -sinT, sinT], axis=0))        # [128, S]

    cc = np.arange(P)[:, None]
    mm = np.arange(896)[None, :]
    stair = (cc > (mm - 384)).astype(np.float32)
    negI = (-BIG * np.eye(P)).astype(np.float32)
    ones1 = np.ones((1, HD), dtype=np.float32)

    import ml_dtypes
    b16 = ml_dtypes.bfloat16

    def _pcf(w_slice, width):
        # [D, width] -> [P, DCH, width]: partition-contiguous weight layout
        a = np.ascontiguousarray(w_slice).reshape(DCH, P, width)
        return np.ascontiguousarray(a.transpose(1, 0, 2))

    in_maps = []
    for b in range(B):
        # x[b].T [D, S] -> [sb, p, c, q] so each s-block DMA is contiguous
        xTb = np.ascontiguousarray(
            x[b].T.reshape(DCH, P, NQB, QB).transpose(2, 1, 0, 3)
        ).astype(b16)
        for g in range(KH):
            wk_g = wk_p[:, g * HD:(g + 1) * HD]
            wk_dup = np.concatenate([wk_g, wk_g], axis=1)       # [D, 128]
            wo_g = wo[g * GH * HD:(g + 1) * GH * HD].reshape(2, P, D)
            m = {
                "xT": xTb,
                "wq": _pcf(wq_p[:, g * GH * HD:(g + 1) * GH * HD],
                           GH * HD).astype(b16),
                "wk": _pcf(wk_dup, 2 * HD).astype(b16),
                "wv": _pcf(wv[:, g * HD:(g + 1) * HD], HD).astype(b16),
                "wo": np.ascontiguousarray(
                    wo_g.transpose(1, 0, 2)).astype(b16),
                "cos": cos_rep.astype(b16),
                "sin": sin_signed.astype(b16),
                "stair": stair.astype(b16),
                "negI": negI.astype(b16),
                "ones1": ones1.astype(b16),
            }
            if mode == "general":
                m["maskT"] = np.ascontiguousarray(
                    mask.reshape(S, S).T).reshape(NKT, P, S)
            in_maps.append(m)
    return in_maps

